# revision 9
# baseline (speedup 1.0000x reference)
"""AttnDecoderRNN single-step on 8 Trainium2 NeuronCores.

Strategy (tensor-parallel over vocab):
- out_w [V,H] dominates memory traffic: rows sharded across 8 cores,
  host-pretransposed to [H, V/8] and cast to bf16 (quarter PE cost, half
  DMA bytes; logp max rel err ~3e-4). PE streams N=512 moving tiles
  against a [128,1] stationary h_new chunk.
- Embedding table sharded by vocab; each core indirect-DMA-gathers the
  token row from its shard (masked to zero off-shard); a point-to-point
  exchange + local sum gives every core `embedded`.
- Attention (enc is 2MB) replicated per-core; attn-combine and the GRU
  cell sharded over their output dim (128 rows/core).
- Cross-core exchanges use XOR-relative remote_dma_broadcast (SBUF to
  SBUF, no collective firmware): slot d delivers to core c ^ SIG[d]
  where SIG = [0,1,2,3,6,7,4,5] (D2D slots land on the diagonal die).
  Receiver block order is therefore a per-core permutation; the host
  pre-permutes each core's weight chunk blocks to compensate, so the
  device never reorders anything.
- log_softmax is distributed: per-core (max, sumexp), one 16-float ncfw
  AllGather (also forces has_collectives -> synchronized core launch),
  local normalization, per-core logp shard written out.

All compute is column-major ([128,1] tiles) so DVE/ACT ops run 128-wide.
"""

import numpy as np
import ml_dtypes

import concourse.bacc as bacc
import concourse.bass as bass
import concourse.mybir as mybir
import concourse.tile as tile
from concourse.bass_utils import run_bass_kernel_spmd

H = 1024
V = 50257
L = 512
NCORES = 8
HC = H // 128            # 8 column-chunks of the hidden dim
VS = 6283                # embedding shard rows (8*6283 = 50264 >= V)
VP = 6400                # out-proj shard rows, padded (8*6400 = 51200)
FP = VP // 128           # 50 columns in the [128, FP] logits layout
PAD_BIAS = -30000.0      # pad logits: exp underflows to 0, max unaffected
SIG = [0, 1, 2, 3, 6, 7, 4, 5]   # measured remote-DMA slot->tpb-xor map

F32 = mybir.dt.float32
BF16 = mybir.dt.bfloat16
I32 = mybir.dt.int32
AX = mybir.AxisListType
AF = mybir.ActivationFunctionType
OP = mybir.AluOpType

# vpack column offsets
VC_H0NAT = 0
VC_H0PERM = 8
VC_H0S = 16
VC_BC = 17
VC_BR = 18
VC_BZ = 19
VC_BNIH = 20
VC_BNHH = 21
VC_OBT = 22
VC_W = 22 + FP

_CACHED_NC = None


def _build():
    nc = bacc.Bacc(None, target_bir_lowering=False, num_devices=NCORES)

    # ---- I/O ----
    ipack_d = nc.dram_tensor("ipack", [2], I32, kind="ExternalInput")
    emb_d = nc.dram_tensor("embs", [VS, H], F32, kind="ExternalInput")
    encp_d = nc.dram_tensor("encp", [128, 4 * H], F32, kind="ExternalInput")
    encTp_d = nc.dram_tensor("encTp", [128, HC * L], F32, kind="ExternalInput")
    wcp_d = nc.dram_tensor("wcp", [128, 2 * H], F32, kind="ExternalInput")
    wihp_d = nc.dram_tensor("wihp", [128, HC * 384], F32, kind="ExternalInput")
    whhp_d = nc.dram_tensor("whhp", [128, HC * 384], F32, kind="ExternalInput")
    vpack_d = nc.dram_tensor("vpack", [128, VC_W], F32, kind="ExternalInput")
    wt_d = nc.dram_tensor("wt", [H, VP], BF16, kind="ExternalInput")

    logp_o = nc.dram_tensor("logp_o", [VP], F32, kind="ExternalOutput")
    hid_o = nc.dram_tensor("hid_o", [128, HC], F32, kind="ExternalOutput")
    attnw_o = nc.dram_tensor("attnw_o", [L], F32, kind="ExternalOutput")

    rg = [list(range(NCORES))]

    with tile.TileContext(nc) as tc:
        with (
            tc.tile_pool(name="const", bufs=1) as cp,
            tc.tile_pool(name="work", bufs=1) as wp,
            tc.tile_pool(name="ps", bufs=1, space="PSUM") as ps,
            tc.tile_pool(name="psw", bufs=3, space="PSUM") as psw,
            tc.tile_pool(name="dram", bufs=1, space="DRAM") as dp,
        ):
            row1 = lambda d: d.rearrange("(a f) -> a f", a=1)

            rsem_e = nc.alloc_semaphore("rsem_e")
            rsem_x = nc.alloc_semaphore("rsem_x")
            rsem_h = nc.alloc_semaphore("rsem_h")
            lsem = nc.alloc_semaphore("lsem")

            def exchange(src, gbuf, w, rsem):
                """XOR all-gather: src [128,w] -> gbuf [128,8*w].

                gbuf block d = src of core (c ^ SIG[d]); block 0 = own.
                """
                nc.vector.tensor_copy(gbuf[:, 0:w], src[:, :])
                for d in range(1, NCORES):
                    rdests = [None] * NCORES
                    rdests[d] = (0, d)
                    nc.gpsimd.remote_dma_broadcast(
                        out_ap=gbuf[:, d * w:(d + 1) * w],
                        in_ap=src[:, :],
                        remote_sem=rsem,
                        local_sem=lsem,
                        rdests=rdests,
                    )
                nc.gpsimd.trigger_dma(count=NCORES - 1)

            # ---- constants ----
            ones128 = cp.tile([1, 128], F32)
            nc.gpsimd.memset(ones128[:, :], 1.0)
            iota8p = cp.tile([8, 1], I32)
            nc.gpsimd.iota(iota8p[:, :], pattern=[[0, 1]], base=0, channel_multiplier=1)
            iota8f = cp.tile([8, 8], I32)
            nc.gpsimd.iota(iota8f[:, :], pattern=[[1, 8]], base=0, channel_multiplier=0)
            iota8f_f = cp.tile([8, 8], F32)
            nc.vector.tensor_copy(iota8f_f[:, :], iota8f[:, :])
            iota8p_f = cp.tile([8, 1], F32)
            nc.vector.tensor_copy(iota8p_f[:, :], iota8p[:, :])
            eye8 = cp.tile([8, 8], F32)
            nc.vector.tensor_scalar(
                out=eye8[:, :], in0=iota8f_f[:, :], scalar1=iota8p_f[:, :1],
                scalar2=None, op0=OP.is_equal,
            )

            # ---- packed input loads ----
            ipack_sb = wp.tile([1, 2], I32)
            nc.sync.dma_start(ipack_sb[:, :], ipack_d.rearrange("(a f) -> a f", a=1))
            vpack = cp.tile([128, VC_W], F32)
            nc.sync.dma_start(vpack[:, :], vpack_d[:, :].rearrange("p f -> p f"))
            enc_sb = cp.tile([128, 4 * H], F32)
            nc.sync.dma_start(enc_sb[:, :], encp_d[:, :].rearrange("p f -> p f"))
            encT_sb = cp.tile([128, HC * L], F32)
            nc.sync.dma_start(encT_sb[:, :], encTp_d[:, :].rearrange("p f -> p f"))
            wc_sb = cp.tile([128, 2 * H], F32)
            nc.sync.dma_start(wc_sb[:, :], wcp_d[:, :].rearrange("p f -> p f"))
            wih_sb = cp.tile([128, HC * 384], F32)
            nc.sync.dma_start(wih_sb[:, :], wihp_d[:, :].rearrange("p f -> p f"))
            whh_sb = cp.tile([128, HC * 384], F32)
            nc.sync.dma_start(whh_sb[:, :], whhp_d[:, :].rearrange("p f -> p f"))

            tok_sb = ipack_sb[:, 0:1]
            cbase_sb = ipack_sb[:, 1:2]
            h0nat = vpack[:, VC_H0NAT:VC_H0NAT + 8]
            h0perm = vpack[:, VC_H0PERM:VC_H0PERM + 8]
            h0s = vpack[:, VC_H0S:VC_H0S + 1]
            bc_sb = vpack[:, VC_BC:VC_BC + 1]
            br_sb = vpack[:, VC_BR:VC_BR + 1]
            bz_sb = vpack[:, VC_BZ:VC_BZ + 1]
            bnih_sb = vpack[:, VC_BNIH:VC_BNIH + 1]
            bnhh_sb = vpack[:, VC_BNHH:VC_BNHH + 1]
            obT_sb = vpack[:, VC_OBT:VC_OBT + FP]

            # ---- out-proj weights: 8 resident bf16 blocks, start loading now ----
            wtblks = []
            for j in range(HC):
                wtb = cp.tile([128, VP], BF16, tag=f"wtb{j}")
                nc.sync.dma_start(wtb[:, :], wt_d[j * 128:(j + 1) * 128, :])
                wtblks.append(wtb)

            # ---- embedding gather (sharded table, masked, P2P sum) ----
            lidx = wp.tile([1, 1], I32)
            nc.vector.tensor_tensor(out=lidx[:, :], in0=tok_sb, in1=cbase_sb, op=OP.subtract)
            nc.vector.tensor_scalar(out=lidx[:, :], in0=lidx[:, :], scalar1=0, scalar2=VS - 1, op0=OP.max, op1=OP.min)
            chi = wp.tile([1, 1], I32)
            nc.vector.tensor_scalar(out=chi[:, :], in0=cbase_sb, scalar1=VS, scalar2=None, op0=OP.add)
            m1 = wp.tile([1, 1], I32)
            nc.vector.tensor_tensor(out=m1[:, :], in0=tok_sb, in1=cbase_sb, op=OP.is_ge)
            m2 = wp.tile([1, 1], I32)
            nc.vector.tensor_tensor(out=m2[:, :], in0=tok_sb, in1=chi[:, :], op=OP.is_lt)
            mask_f = wp.tile([1, 1], F32)
            nc.vector.tensor_tensor(out=mask_f[:, :], in0=m1[:, :], in1=m2[:, :], op=OP.mult)
            lidx_f = wp.tile([1, 1], F32)
            nc.vector.tensor_copy(lidx_f[:, :], lidx[:, :])
            ridx_ps = ps.tile([8, 1], F32, tag="pss", bufs=3)
            nc.tensor.matmul(ridx_ps[:, :], ones128[:1, :8], lidx_f[:, :], start=True, stop=True)
            ridx = wp.tile([8, 1], I32)
            nc.vector.tensor_copy(ridx[:, :], ridx_ps[:, :])
            nc.vector.tensor_scalar(out=ridx[:, :], in0=ridx[:, :], scalar1=8, scalar2=None, op0=OP.mult)
            nc.vector.tensor_tensor(out=ridx[:, :], in0=ridx[:, :], in1=iota8p[:, :], op=OP.add)
            gath = wp.tile([8, 128], F32)
            nc.gpsimd.indirect_dma_start(
                out=gath[:, :], out_offset=None,
                in_=emb_d[:, :].rearrange("v (a f) -> (v a) f", a=8),
                in_offset=bass.IndirectOffsetOnAxis(ap=ridx[:, :1], axis=0),
            )
            embT_ps = ps.tile([128, 8], F32, tag="pss", bufs=3)
            nc.tensor.matmul(embT_ps[:, :], gath[:, :], eye8[:, :], start=True, stop=True)
            maskb_ps = ps.tile([128, 1], F32, tag="pss", bufs=3)
            nc.tensor.matmul(maskb_ps[:, :], ones128[:, :], mask_f[:, :], start=True, stop=True)
            maskb = wp.tile([128, 1], F32)
            nc.vector.tensor_copy(maskb[:, :], maskb_ps[:, :])
            embm = wp.tile([128, 8], F32)
            nc.vector.tensor_scalar(out=embm[:, :], in0=embT_ps[:, :], scalar1=maskb[:, :1], scalar2=None, op0=OP.mult)

            gbufE = wp.tile([128, 64], F32)
            exchange(embm, gbufE, 8, rsem_e)
            embcols = wp.tile([128, 8], F32)
            with tc.tile_critical():
                nc.vector.wait_ge(rsem_e, 14)
                nc.vector.tensor_reduce(
                    embcols[:, :], gbufE[:, :].rearrange("p (s c) -> p c s", s=8),
                    axis=AX.X, op=OP.add,
                )

            # ---- attention (replicated) ----
            scores_ps = ps.tile([1, L], F32, tag="pss", bufs=3)
            for k in range(HC):
                nc.tensor.matmul(scores_ps[:, :], h0nat[:, k:k + 1], encT_sb[:, k * L:(k + 1) * L],
                                 start=(k == 0), stop=(k == HC - 1))
            smax = wp.tile([1, 1], F32)
            nc.vector.reduce_max(smax[:, :], scores_ps[:, :], axis=AX.X)
            nsmax = wp.tile([1, 1], F32)
            nc.vector.tensor_scalar(out=nsmax[:, :], in0=smax[:, :], scalar1=-1.0, scalar2=None, op0=OP.mult)
            attnw_e = wp.tile([1, L], F32)
            ssum = wp.tile([1, 1], F32)
            nc.scalar.activation(attnw_e[:, :], scores_ps[:, :], AF.Exp, bias=nsmax[:, :1], accum_out=ssum[:, :1])
            rcp = wp.tile([1, 1], F32)
            nc.vector.reciprocal(rcp[:, :], ssum[:, :])
            attnw = wp.tile([1, L], F32)
            nc.vector.tensor_scalar(out=attnw[:, :], in0=attnw_e[:, :], scalar1=rcp[:, :1], scalar2=None, op0=OP.mult)
            nc.sync.dma_start(row1(attnw_o), attnw[:, :])

            awT_ps = ps.tile([128, 4], F32, tag="pss", bufs=3)
            for j in range(4):
                nc.tensor.matmul(awT_ps[:, j:j + 1], attnw[:1, j * 128:(j + 1) * 128], eye8[:1, :1],
                                 start=True, stop=True)
            awcols = wp.tile([128, 4], F32)
            nc.vector.tensor_copy(awcols[:, :], awT_ps[:, :])

            applT_ps = ps.tile([128, HC], F32, tag="pss", bufs=3)
            for hb in range(HC):
                for j in range(4):
                    nc.tensor.matmul(applT_ps[:, hb:hb + 1],
                                     enc_sb[:, j * H + hb * 128: j * H + (hb + 1) * 128],
                                     awcols[:, j:j + 1], start=(j == 0), stop=(j == 3))
            applcols = wp.tile([128, HC], F32)
            nc.vector.tensor_copy(applcols[:, :], applT_ps[:, :])

            # ---- attn_combine + relu (output shard [128,1]) ----
            xcT_ps = ps.tile([128, 1], F32, tag="pss", bufs=3)
            for k in range(16):
                src = embcols if k < 8 else applcols
                nc.tensor.matmul(xcT_ps[:, :], wc_sb[:, k * 128:(k + 1) * 128],
                                 src[:, (k % 8):(k % 8) + 1], start=(k == 0), stop=(k == 15))
            xcT = wp.tile([128, 1], F32)
            nc.scalar.activation(xcT[:, :], xcT_ps[:, :], AF.Relu, bias=bc_sb)

            gbufX = wp.tile([128, 8], F32)
            exchange(xcT, gbufX, 1, rsem_x)
            xcols = wp.tile([128, 8], F32)
            with tc.tile_critical():
                nc.vector.wait_ge(rsem_x, 14)
                nc.vector.tensor_copy(xcols[:, :], gbufX[:, :])

            # ---- GRU cell (output shard [128,1]; k-blocks in SIG_c order) ----
            giT_ps = ps.tile([128, 3], F32, tag="pss", bufs=3)
            ghT_ps = ps.tile([128, 3], F32, tag="pss", bufs=3)
            for g in range(3):
                for k in range(HC):
                    nc.tensor.matmul(giT_ps[:, g:g + 1], wih_sb[:, k * 384 + g * 128: k * 384 + (g + 1) * 128],
                                     xcols[:, k:k + 1], start=(k == 0), stop=(k == HC - 1))
            for g in range(3):
                for k in range(HC):
                    nc.tensor.matmul(ghT_ps[:, g:g + 1], whh_sb[:, k * 384 + g * 128: k * 384 + (g + 1) * 128],
                                     h0perm[:, k:k + 1], start=(k == 0), stop=(k == HC - 1))
            gi = wp.tile([128, 3], F32)
            nc.vector.tensor_copy(gi[:, :], giT_ps[:, :])
            gh = wp.tile([128, 3], F32)
            nc.vector.tensor_copy(gh[:, :], ghT_ps[:, :])
            prz = wp.tile([128, 2], F32)
            nc.vector.tensor_add(prz[:, :], gi[:, 0:2], gh[:, 0:2])
            r_g = wp.tile([128, 1], F32)
            nc.scalar.activation(r_g[:, :], prz[:, 0:1], AF.Sigmoid, bias=br_sb)
            z_g = wp.tile([128, 1], F32)
            nc.scalar.activation(z_g[:, :], prz[:, 1:2], AF.Sigmoid, bias=bz_sb)
            ghn = wp.tile([128, 1], F32)
            nc.vector.tensor_add(ghn[:, :], gh[:, 2:3], bnhh_sb)
            rghn = wp.tile([128, 1], F32)
            nc.vector.tensor_mul(rghn[:, :], r_g[:, :], ghn[:, :])
            npre = wp.tile([128, 1], F32)
            nc.vector.tensor_add(npre[:, :], gi[:, 2:3], rghn[:, :])
            n_g = wp.tile([128, 1], F32)
            nc.scalar.activation(n_g[:, :], npre[:, :], AF.Tanh, bias=bnih_sb)
            dvec = wp.tile([128, 1], F32)
            nc.vector.tensor_tensor(out=dvec[:, :], in0=h0s, in1=n_g[:, :], op=OP.subtract)
            zd = wp.tile([128, 1], F32)
            nc.vector.tensor_mul(zd[:, :], z_g[:, :], dvec[:, :])
            hnT = wp.tile([128, 1], F32)
            nc.vector.tensor_add(hnT[:, :], n_g[:, :], zd[:, :])

            gbufH = wp.tile([128, 8], F32)
            exchange(hnT, gbufH, 1, rsem_h)
            hnb = wp.tile([128, 8], BF16)
            hnf = wp.tile([128, 8], F32)
            with tc.tile_critical():
                nc.vector.wait_ge(rsem_h, 14)
                nc.vector.tensor_copy(hnb[:, :], gbufH[:, :])
                nc.vector.tensor_copy(hnf[:, :], gbufH[:, :])
            # raw (SIG_c-permuted) h_new chunks; host un-permutes
            nc.sync.dma_start(hid_o[:, :], hnf[:, :])

            # ---- out projection: resident bf16 blocks, [1,512] psum matvecs ----
            lg_stage = dp.tile([VP], F32)
            vts = [(i * 512, 512) for i in range(VP // 512)]
            if VP % 512:
                vts.append((VP - VP % 512, VP % 512))
            for v0, w in vts:
                lg_ps = psw.tile([1, 512], F32, tag="lg")
                for j in range(HC):
                    nc.tensor.matmul(lg_ps[:, :w], hnb[:, j:j + 1], wtblks[j][:, v0:v0 + w],
                                     start=(j == 0), stop=(j == HC - 1))
                lg_sb = wp.tile([1, 512], F32, tag="lg_sb", bufs=3)
                nc.vector.tensor_copy(lg_sb[:, :w], lg_ps[:, :w])
                nc.sync.dma_start(lg_stage[v0:v0 + w], lg_sb[:1, :w])

            # ---- distributed log_softmax ----
            lg128 = wp.tile([128, FP], F32)
            nc.sync.dma_start(lg128[:, :], lg_stage.rearrange("(f p) -> p f", p=128))
            l_sb = wp.tile([128, FP], F32)
            nc.vector.tensor_add(l_sb[:, :], lg128[:, :], obT_sb)
            mp = wp.tile([128, 1], F32)
            nc.vector.reduce_max(mp[:, :], l_sb[:, :], axis=AX.X)
            m_loc = wp.tile([1, 1], F32)
            nc.gpsimd.tensor_reduce(m_loc[:, :], mp[:, :], axis=AX.C, op=OP.max)
            nm = wp.tile([1, 1], F32)
            nc.vector.tensor_scalar(out=nm[:, :], in0=m_loc[:, :], scalar1=-1.0, scalar2=None, op0=OP.mult)
            nmb_ps = ps.tile([128, 1], F32, tag="pss", bufs=3)
            nc.tensor.matmul(nmb_ps[:, :], ones128[:, :], nm[:, :], start=True, stop=True)
            nmb = wp.tile([128, 1], F32)
            nc.vector.tensor_copy(nmb[:, :], nmb_ps[:, :])
            e_sb = wp.tile([128, FP], F32)
            sp = wp.tile([128, 1], F32)
            nc.scalar.activation(e_sb[:, :], l_sb[:, :], AF.Exp, bias=nmb[:, :1], accum_out=sp[:, :1])
            s_loc = wp.tile([1, 1], F32)
            nc.gpsimd.tensor_reduce(s_loc[:, :], sp[:, :], axis=AX.C, op=OP.add)
            st = wp.tile([1, 2], F32)
            nc.vector.tensor_copy(st[:, 0:1], m_loc[:, :])
            nc.vector.tensor_copy(st[:, 1:2], s_loc[:, :])
            ags_in = dp.tile([2], F32)
            ags_out = dp.tile([2 * NCORES], F32)
            nc.sync.dma_start(row1(ags_in), st[:, :])
            nc.gpsimd.collective_compute(
                "AllGather", OP.bypass, replica_groups=rg,
                ins=[ags_in[:].opt()], outs=[ags_out[:].opt()],
            )
            mv = wp.tile([1, NCORES], F32)
            sv = wp.tile([1, NCORES], F32)
            ags_v = ags_out.rearrange("(c s) -> s c", s=2)
            nc.sync.dma_start(mv[:, :], ags_v[0:1, :])
            nc.sync.dma_start(sv[:, :], ags_v[1:2, :])
            mg = wp.tile([1, 1], F32)
            nc.vector.reduce_max(mg[:, :], mv[:, :], axis=AX.X)
            dm = wp.tile([1, NCORES], F32)
            nc.vector.tensor_scalar(out=dm[:, :], in0=mv[:, :], scalar1=mg[:, :1], scalar2=None, op0=OP.subtract)
            em = wp.tile([1, NCORES], F32)
            nc.scalar.activation(em[:, :], dm[:, :], AF.Exp)
            pr = wp.tile([1, NCORES], F32)
            nc.vector.tensor_mul(pr[:, :], em[:, :], sv[:, :])
            sg = wp.tile([1, 1], F32)
            nc.vector.reduce_sum(sg[:, :], pr[:, :], axis=AX.X)
            lng = wp.tile([1, 1], F32)
            nc.scalar.activation(lng[:, :], sg[:, :], AF.Ln)
            nlse = wp.tile([1, 1], F32)
            nc.vector.tensor_add(nlse[:, :], mg[:, :], lng[:, :])
            nc.vector.tensor_scalar(out=nlse[:, :], in0=nlse[:, :], scalar1=-1.0, scalar2=None, op0=OP.mult)
            nlb_ps = ps.tile([128, 1], F32, tag="pss", bufs=3)
            nc.tensor.matmul(nlb_ps[:, :], ones128[:, :], nlse[:, :], start=True, stop=True)
            nlb = wp.tile([128, 1], F32)
            nc.vector.tensor_copy(nlb[:, :], nlb_ps[:, :])
            logp_sb = wp.tile([128, FP], F32)
            nc.vector.tensor_scalar(out=logp_sb[:, :], in0=l_sb[:, :], scalar1=nlb[:, :1],
                                    scalar2=None, op0=OP.add)
            nc.sync.dma_start(logp_o.rearrange("(f p) -> p f", p=128), logp_sb[:, :])

    nc.compile()
    return nc


def _get_nc():
    global _CACHED_NC
    if _CACHED_NC is None:
        _CACHED_NC = _build()
    return _CACHED_NC


def kernel(input_tok, hidden, encoder_outputs, embedding_w,
           attn_combine_w, attn_combine_b, w_ih, w_hh, b_ih, b_hh,
           out_w, out_b):
    nc = _get_nc()

    tok = int(np.asarray(input_tok).reshape(-1)[0])  # only packed as data below
    h0 = np.asarray(hidden, dtype=np.float32).reshape(1, H)
    enc = np.ascontiguousarray(np.asarray(encoder_outputs, dtype=np.float32).reshape(L, H))
    encT = np.ascontiguousarray(enc.T)
    emb = np.asarray(embedding_w, dtype=np.float32)
    wc = np.asarray(attn_combine_w, dtype=np.float32)
    bc = np.asarray(attn_combine_b, dtype=np.float32)
    wih = np.asarray(w_ih, dtype=np.float32)
    whh = np.asarray(w_hh, dtype=np.float32)
    bi = np.asarray(b_ih, dtype=np.float32).reshape(3, HC, 128)
    bh = np.asarray(b_hh, dtype=np.float32).reshape(3, HC, 128)
    ow = np.asarray(out_w, dtype=np.float32)
    ob = np.asarray(out_b, dtype=np.float32)

    emb_pad = np.zeros((NCORES * VS, H), dtype=np.float32)
    emb_pad[:V] = emb
    w_pad = np.zeros((NCORES * VP, H), dtype=np.float32)
    w_pad[:V] = ow
    b_pad = np.full(NCORES * VP, PAD_BIAS, dtype=np.float32)
    b_pad[:V] = ob
    h0cols = np.ascontiguousarray(h0.reshape(HC, 128).T)          # [128, 8] natural
    # packed tile layouts: [n,128,F] blocks -> [128, n*F]
    pack = lambda a, n: np.ascontiguousarray(
        a.reshape(n, 128, -1).transpose(1, 0, 2).reshape(128, -1))
    encp = pack(enc, 4)
    encTp = pack(encT, HC)

    in_maps = []
    for c in range(NCORES):
        sl = slice(c * 128, (c + 1) * 128)
        perm = [c ^ SIG[j] for j in range(HC)]                     # block j <- chunk perm[j]
        wih_c = np.concatenate([wih[g * H + c * 128: g * H + (c + 1) * 128] for g in range(3)])
        whh_c = np.concatenate([whh[g * H + c * 128: g * H + (c + 1) * 128] for g in range(3)])
        wihT_c = np.ascontiguousarray(wih_c.T).reshape(HC, 128, 384)
        whhT_c = np.ascontiguousarray(whh_c.T).reshape(HC, 128, 384)
        wihp = pack(wihT_c[perm].reshape(HC * 128, 384), HC)
        whhp = pack(whhT_c[perm].reshape(HC * 128, 384), HC)
        wtT_c = np.ascontiguousarray(w_pad[c * VP:(c + 1) * VP].T).reshape(HC, 128, VP)
        wt_c = wtT_c[perm].reshape(H, VP).astype(ml_dtypes.bfloat16)
        vpack = np.zeros((128, VC_W), dtype=np.float32)
        vpack[:, VC_H0NAT:VC_H0NAT + 8] = h0cols
        vpack[:, VC_H0PERM:VC_H0PERM + 8] = h0cols[:, perm]
        vpack[:, VC_H0S] = h0[0, sl]
        vpack[:, VC_BC] = bc[sl]
        vpack[:, VC_BR] = bi[0, c] + bh[0, c]
        vpack[:, VC_BZ] = bi[1, c] + bh[1, c]
        vpack[:, VC_BNIH] = bi[2, c]
        vpack[:, VC_BNHH] = bh[2, c]
        vpack[:, VC_OBT:VC_OBT + FP] = b_pad[c * VP:(c + 1) * VP].reshape(FP, 128).T
        in_maps.append({
            "ipack": np.array([tok, c * VS], dtype=np.int32),
            "embs": emb_pad[c * VS:(c + 1) * VS],
            "encp": encp,
            "encTp": encTp,
            "wcp": pack(np.ascontiguousarray(wc[sl, :].T), 16),
            "wihp": wihp,
            "whhp": whhp,
            "vpack": vpack,
            "wt": wt_c,
        })

    global _last_in_maps
    _last_in_maps = in_maps
    res = run_bass_kernel_spmd(nc, in_maps, core_ids=list(range(NCORES)))

    logp = np.concatenate([res.results[c]["logp_o"] for c in range(NCORES)])[:V]
    logp = logp.reshape(1, V).astype(np.float32)
    hid_raw = res.results[0]["hid_o"]                              # [128, 8], block j = chunk SIG[j]
    h_new = np.empty((HC, 128), dtype=np.float32)
    for j in range(HC):
        h_new[SIG[j]] = hid_raw[:, j]
    h_new = h_new.reshape(1, 1, H)
    attnw = res.results[0]["attnw_o"].reshape(1, L).astype(np.float32)
    return logp, h_new, attnw


# revision 14
# speedup vs baseline: 1.2820x; 1.2820x over previous
"""AttnDecoderRNN single-step on 8 Trainium2 NeuronCores.

Tensor-parallel over vocab for the dominant out-projection, with the
small attention/GRU state computed cooperatively:

- out_w [V,H]: rows sharded across 8 cores, host-pretransposed to
  [H, V/8] bf16 (logp max rel err ~3e-4). PE streams N=512 moving tiles
  against [128,1] stationary h_new chunks.
- Embedding table sharded by HIDDEN dim: each core indirect-DMA-gathers
  the token row slice [1,128] from its own [V,128] shard, multiplies by
  its slice of attn_combine's embedding half -> partial pre-activation
  E_c [128,8]; one cross-core sum-exchange yields the full embedding
  contribution with no owner logic.
- attn_applied's combine half uses a replicated [1024,1024] weight, so
  x = relu(E + A + b) is fully local on every core; the GRU is sharded
  over its output dim (128 rows/core), and only h_new needs a second
  exchange.
- Cross-core exchanges use XOR-relative remote_dma_broadcast (SBUF to
  SBUF, no collective firmware): slot d delivers to core c ^ SIG[d],
  SIG = [0,1,2,3,6,7,4,5] (D2D slots land on the diagonal die).
  Descriptors are pre-generated at kernel start; the SWDGE trigger is
  gated on data readiness inside a critical section. The h_new receive
  order is a per-core permutation; the host pre-permutes each core's
  out_w chunk blocks to compensate.
- log_softmax is distributed: per-core (max, sumexp), one 16-float ncfw
  AllGather, local normalization. A dummy AllGather issued at t=0 warms
  the collective stream (entry barrier + first-op cost overlap the main
  DMA/compute) and forces has_collectives -> synchronized core launch.
"""

import numpy as np
import ml_dtypes

import concourse.bacc as bacc
import concourse.bass as bass
import concourse.mybir as mybir
import concourse.tile as tile
from concourse.bass_utils import run_bass_kernel_spmd

H = 1024
V = 50257
L = 512
NCORES = 8
HC = H // 128            # 8 column-chunks of the hidden dim
VP = 6400                # out-proj shard rows, padded (8*6400 = 51200)
FP = VP // 128           # 50 columns in the [128, FP] logits layout
PAD_BIAS = -30000.0      # pad logits: exp underflows to 0, max unaffected
SIG = [0, 1, 2, 3, 6, 7, 4, 5]   # measured remote-DMA slot->tpb-xor map

F32 = mybir.dt.float32
BF16 = mybir.dt.bfloat16
I32 = mybir.dt.int32
AX = mybir.AxisListType
AF = mybir.ActivationFunctionType
OP = mybir.AluOpType

# vpack column offsets (per-partition data)
VC_H0 = 0      # h0cols natural [128,8]
VC_BCC = 8     # attn_combine bias, column form [128,8]
VC_OBT = 16    # out_b shard, [128,50] p-major
VC_W = 16 + FP
# rpack column offsets (row data, partition 0)
RC_BRZ = 0     # b_ih+b_hh for r,z [256]
RC_BNIH = 256
RC_BNHH = 384
RC_H0S = 512   # h0 slice for this core's GRU rows [128]
RC_W = 640

_CACHED_NC = None


def _build():
    nc = bacc.Bacc(None, target_bir_lowering=False, num_devices=NCORES)

    ipack_d = nc.dram_tensor("ipack", [1], I32, kind="ExternalInput")
    emb_d = nc.dram_tensor("embs", [V, 128], F32, kind="ExternalInput")
    encTp_d = nc.dram_tensor("encTp", [128, HC * L], F32, kind="ExternalInput")
    wcep_d = nc.dram_tensor("wcep", [128, H], F32, kind="ExternalInput")
    wcap_d = nc.dram_tensor("wcap", [128, HC * H], F32, kind="ExternalInput")
    wihp_d = nc.dram_tensor("wihp", [128, HC * 384], F32, kind="ExternalInput")
    whhp_d = nc.dram_tensor("whhp", [128, HC * 384], F32, kind="ExternalInput")
    vpack_d = nc.dram_tensor("vpack", [128, VC_W], F32, kind="ExternalInput")
    rpack_d = nc.dram_tensor("rpack", [RC_W], F32, kind="ExternalInput")
    wt_d = nc.dram_tensor("wt", [H, VP], BF16, kind="ExternalInput")

    logp_o = nc.dram_tensor("logp_o", [128, FP], F32, kind="ExternalOutput")
    hid_o = nc.dram_tensor("hid_o", [128, HC], F32, kind="ExternalOutput")
    attnw_o = nc.dram_tensor("attnw_o", [L], F32, kind="ExternalOutput")

    rg = [list(range(NCORES))]

    with tile.TileContext(nc) as tc:
        with (
            tc.tile_pool(name="const", bufs=1) as cp,
            tc.tile_pool(name="work", bufs=1) as wp,
            tc.tile_pool(name="ps", bufs=1, space="PSUM") as ps,
            tc.tile_pool(name="psw", bufs=2, space="PSUM") as psw,
            tc.tile_pool(name="dram", bufs=1, space="DRAM") as dp,
        ):
            row1 = lambda d: d.rearrange("(a f) -> a f", a=1)

            rsem_e = nc.alloc_semaphore("rsem_e")
            rsem_h = nc.alloc_semaphore("rsem_h")
            lsem = nc.alloc_semaphore("lsem")

            # dummy collective first: warms the ncfw stream concurrently
            # with the main compute so the real stats AllGather is cheap
            dum_in = dp.tile([2], F32)
            dum_out = dp.tile([2 * NCORES], F32)
            nc.gpsimd.collective_compute(
                "AllGather", OP.bypass, replica_groups=rg,
                ins=[dum_in[:].opt()], outs=[dum_out[:].opt()],
            )

            # exchange buffers; descriptors are pre-generated right after the
            # indirect gather (the only other SWDGE ring user) below
            E_sb = wp.tile([128, 8], F32)
            gbufE = wp.tile([128, 64], F32)
            hT_sb = wp.tile([128, 1], F32)
            gbufH = wp.tile([128, 8], F32)

            # ---- constants ----
            ones2 = cp.tile([1, 2], F32)
            nc.gpsimd.memset(ones2[:, :], 1.0)
            one1 = cp.tile([1, 1], F32)
            nc.gpsimd.memset(one1[:, :], 1.0)
            ones128 = cp.tile([1, 128], F32)
            nc.gpsimd.memset(ones128[:, :], 1.0)

            # ---- packed input loads ----
            ipack_sb = wp.tile([1, 1], I32)
            nc.sync.dma_start(ipack_sb[:, :], row1(ipack_d))
            vpack = cp.tile([128, VC_W], F32)
            nc.sync.dma_start(vpack[:, :], vpack_d[:, :].rearrange("p f -> p f"))
            rpack = cp.tile([1, RC_W], F32)
            nc.sync.dma_start(rpack[:, :], row1(rpack_d))
            wcep = cp.tile([128, H], F32)
            nc.sync.dma_start(wcep[:, :], wcep_d[:, :].rearrange("p f -> p f"))
            encT_sb = cp.tile([128, HC * L], F32)
            nc.sync.dma_start(encT_sb[:, :], encTp_d[:, :].rearrange("p f -> p f"))
            wcap = cp.tile([128, HC * H], F32)
            nc.sync.dma_start(wcap[:, :], wcap_d[:, :].rearrange("p f -> p f"))
            wih_sb = cp.tile([128, HC * 384], F32)
            nc.sync.dma_start(wih_sb[:, :], wihp_d[:, :].rearrange("p f -> p f"))
            whh_sb = cp.tile([128, HC * 384], F32)
            nc.sync.dma_start(whh_sb[:, :], whhp_d[:, :].rearrange("p f -> p f"))

            h0nat = vpack[:, VC_H0:VC_H0 + 8]
            bcc = vpack[:, VC_BCC:VC_BCC + 8]
            obT_sb = vpack[:, VC_OBT:VC_OBT + FP]

            # ---- out-proj weights: 8 resident bf16 blocks ----
            wtblks = []
            for j in range(HC):
                wtb = cp.tile([128, VP], BF16, tag=f"wtb{j}", name=f"wtb{j}")
                nc.sync.dma_start(wtb[:, :], wt_d[j * 128:(j + 1) * 128, :])
                wtblks.append(wtb)

            # ---- embedding slice gather + E contribution ----
            tok_f = wp.tile([1, 1], F32)
            nc.vector.tensor_copy(tok_f[:, :], ipack_sb[:, :])
            ridx_ps = ps.tile([2, 1], F32, tag="pss", bufs=3)
            nc.tensor.matmul(ridx_ps[:, :], ones2[:1, :2], tok_f[:, :], start=True, stop=True)
            ridx = wp.tile([2, 1], I32)
            nc.vector.tensor_copy(ridx[:, :], ridx_ps[:, :])
            gath = wp.tile([2, 128], F32)
            nc.gpsimd.indirect_dma_start(
                out=gath[:, :], out_offset=None, in_=emb_d[:, :],
                in_offset=bass.IndirectOffsetOnAxis(ap=ridx[:, :1], axis=0),
            )
            embT_ps = ps.tile([128, 1], F32, tag="pss", bufs=3)
            nc.tensor.matmul(embT_ps[:, :], gath[:1, :], one1[:, :], start=True, stop=True)
            embT = wp.tile([128, 1], F32)
            nc.vector.tensor_copy(embT[:, :], embT_ps[:, :])
            E_ps = ps.tile([128, 8], F32, tag="pss", bufs=3)
            for j in range(HC):
                nc.tensor.matmul(E_ps[:, j:j + 1], wcep[:, j * 128:(j + 1) * 128],
                                 embT[:, :], start=True, stop=True)
            nc.vector.tensor_copy(E_sb[:, :], E_ps[:, :])
            for d in range(1, NCORES):
                rdests = [None] * NCORES
                rdests[d] = (0, d)
                nc.gpsimd.remote_dma_broadcast(
                    out_ap=gbufE[:, d * 8:(d + 1) * 8], in_ap=E_sb[:, :],
                    remote_sem=rsem_e, local_sem=lsem, rdests=rdests)
            nc.gpsimd.trigger_dma(count=NCORES - 1)
            nc.vector.tensor_copy(gbufE[:, 0:8], E_sb[:, :])

            # ---- attention (replicated) ----
            scores_ps = ps.tile([1, L], F32, tag="pss", bufs=3)
            for k in range(HC):
                nc.tensor.matmul(scores_ps[:, :], h0nat[:, k:k + 1], encT_sb[:, k * L:(k + 1) * L],
                                 start=(k == 0), stop=(k == HC - 1))
            smax = wp.tile([1, 1], F32)
            nc.vector.reduce_max(smax[:, :], scores_ps[:, :], axis=AX.X)
            nsmax = wp.tile([1, 1], F32)
            nc.vector.tensor_scalar(out=nsmax[:, :], in0=smax[:, :], scalar1=-1.0, scalar2=None, op0=OP.mult)
            attnw_e = wp.tile([1, L], F32)
            ssum = wp.tile([1, 1], F32)
            nc.scalar.activation(attnw_e[:, :], scores_ps[:, :], AF.Exp, bias=nsmax[:, :1], accum_out=ssum[:, :1])
            rcp = wp.tile([1, 1], F32)
            nc.vector.reciprocal(rcp[:, :], ssum[:, :])
            attnw = attnw_e
            nc.vector.tensor_scalar(out=attnw[:, :], in0=attnw_e[:, :], scalar1=rcp[:, :1], scalar2=None, op0=OP.mult)
            nc.sync.dma_start(row1(attnw_o), attnw[:, :])

            # attn_applied via DVE: broadcast attnw across partitions once,
            # then per h-chunk multiply encT rows and reduce over L
            aw128_ps = ps.tile([128, L], F32, tag="psA", bufs=1)
            nc.tensor.matmul(aw128_ps[:, :], ones128[:, :], attnw[:1, :], start=True, stop=True)
            aw128 = wp.tile([128, L], F32)
            nc.vector.tensor_copy(aw128[:, :], aw128_ps[:, :])
            applcols = wp.tile([128, HC], F32)
            awprod = wp.tile([128, L], F32)
            for k in range(HC):
                nc.vector.tensor_mul(awprod[:, :], encT_sb[:, k * L:(k + 1) * L], aw128[:, :])
                nc.vector.reduce_sum(applcols[:, k:k + 1], awprod[:, :], axis=AX.X)

            # A contribution [128,8]: full combine of the appl half
            A_ps = ps.tile([128, HC], F32, tag="psA", bufs=1)
            for j in range(HC):
                for k in range(HC):
                    nc.tensor.matmul(A_ps[:, j:j + 1],
                                     wcap[:, (k * HC + j) * 128:(k * HC + j) * 128 + 128],
                                     applcols[:, k:k + 1], start=(k == 0), stop=(k == HC - 1))

            # ---- x = relu(E_sum + A + bc), fully local ----
            Esum = wp.tile([128, 8], F32)
            with tc.tile_critical():
                nc.vector.wait_ge(rsem_e, 14)
                nc.vector.tensor_reduce(
                    Esum[:, :], gbufE[:, :].rearrange("p (s c) -> p c s", s=8),
                    axis=AX.X, op=OP.add,
                )
            xpre = wp.tile([128, 8], F32)
            nc.vector.tensor_add(xpre[:, :], Esum[:, :], A_ps[:, :])
            nc.vector.tensor_add(xpre[:, :], xpre[:, :], bcc)
            xcols = wp.tile([128, 8], F32)
            nc.scalar.activation(xcols[:, :], xpre[:, :], AF.Relu)

            # ---- GRU (row form, output shard [1,128]) ----
            gh_ps = ps.tile([1, 384], F32, tag="psg", bufs=2)
            for k in range(HC):
                nc.tensor.matmul(gh_ps[:, :], h0nat[:, k:k + 1], whh_sb[:, k * 384:(k + 1) * 384],
                                 start=(k == 0), stop=(k == HC - 1))
            gh_sb = wp.tile([1, 384], F32)
            nc.vector.tensor_copy(gh_sb[:, :], gh_ps[:, :])
            gi_ps = ps.tile([1, 384], F32, tag="psg", bufs=2)
            for k in range(HC):
                nc.tensor.matmul(gi_ps[:, :], xcols[:, k:k + 1], wih_sb[:, k * 384:(k + 1) * 384],
                                 start=(k == 0), stop=(k == HC - 1))
            gi_sb = wp.tile([1, 384], F32)
            nc.vector.tensor_copy(gi_sb[:, :], gi_ps[:, :])

            rz1 = wp.tile([1, 256], F32)
            nc.vector.tensor_add(rz1[:, :], gi_sb[:, 0:256], gh_sb[:, 0:256])
            nc.vector.tensor_add(rz1[:, :], rz1[:, :], rpack[:, RC_BRZ:RC_BRZ + 256])
            rz = wp.tile([1, 256], F32)
            nc.scalar.activation(rz[:, :], rz1[:, :], AF.Sigmoid)
            ghn = wp.tile([1, 128], F32)
            nc.vector.tensor_add(ghn[:, :], gh_sb[:, 256:384], rpack[:, RC_BNHH:RC_BNHH + 128])
            rghn = wp.tile([1, 128], F32)
            nc.vector.tensor_mul(rghn[:, :], rz[:, 0:128], ghn[:, :])
            npre = wp.tile([1, 128], F32)
            nc.vector.tensor_add(npre[:, :], gi_sb[:, 256:384], rpack[:, RC_BNIH:RC_BNIH + 128])
            nc.vector.tensor_add(npre[:, :], npre[:, :], rghn[:, :])
            n_g = wp.tile([1, 128], F32)
            nc.scalar.activation(n_g[:, :], npre[:, :], AF.Tanh)
            dvec = wp.tile([1, 128], F32)
            nc.vector.tensor_tensor(out=dvec[:, :], in0=rpack[:, RC_H0S:RC_H0S + 128], in1=n_g[:, :], op=OP.subtract)
            zd = wp.tile([1, 128], F32)
            nc.vector.tensor_mul(zd[:, :], rz[:, 128:256], dvec[:, :])
            hrow = wp.tile([1, 128], F32)
            nc.vector.tensor_add(hrow[:, :], n_g[:, :], zd[:, :])
            hT_ps = ps.tile([128, 1], F32, tag="pss", bufs=3)
            nc.tensor.matmul(hT_ps[:, :], hrow[:1, :], one1[:, :], start=True, stop=True)
            nc.vector.tensor_copy(hT_sb[:, :], hT_ps[:, :])
            for d in range(1, NCORES):
                rdests = [None] * NCORES
                rdests[d] = (0, d)
                nc.gpsimd.remote_dma_broadcast(
                    out_ap=gbufH[:, d:d + 1], in_ap=hT_sb[:, :],
                    remote_sem=rsem_h, local_sem=lsem, rdests=rdests)
            nc.gpsimd.trigger_dma(count=NCORES - 1)
            nc.vector.tensor_copy(gbufH[:, 0:1], hT_sb[:, :])

            hnb = wp.tile([128, 8], BF16)
            hnf = wp.tile([128, 8], F32)
            with tc.tile_critical():
                nc.vector.wait_ge(rsem_h, 14)
                nc.vector.tensor_copy(hnb[:, :], gbufH[:, :])
                nc.vector.tensor_copy(hnf[:, :], gbufH[:, :])
            # raw (SIG_c-permuted) h_new chunks; host un-permutes
            nc.sync.dma_start(hid_o[:, :], hnf[:, :])

            # ---- out projection ----
            lg_stage = dp.tile([VP], F32)
            vts = [(i * 512, 512) for i in range(VP // 512)]
            if VP % 512:
                vts.append((VP - VP % 512, VP % 512))
            for v0, w in vts:
                lg_ps = psw.tile([1, 512], F32, tag="lg")
                for j in range(HC):
                    nc.tensor.matmul(lg_ps[:, :w], hnb[:, j:j + 1], wtblks[j][:, v0:v0 + w],
                                     start=(j == 0), stop=(j == HC - 1))
                lg_sb = wp.tile([1, 512], F32, tag="lg_sb", bufs=2)
                nc.vector.tensor_copy(lg_sb[:, :w], lg_ps[:, :w])
                nc.sync.dma_start(lg_stage[v0:v0 + w], lg_sb[:1, :w])

            # ---- distributed log_softmax (p-major layout: v = p*FP + f) ----
            lg128 = wp.tile([128, FP], F32)
            nc.sync.dma_start(lg128[:, :], lg_stage.rearrange("(p f) -> p f", p=128))
            l_sb = wp.tile([128, FP], F32)
            nc.vector.tensor_add(l_sb[:, :], lg128[:, :], obT_sb)
            mp = wp.tile([128, 1], F32)
            nc.vector.reduce_max(mp[:, :], l_sb[:, :], axis=AX.X)
            m_loc = wp.tile([1, 1], F32)
            nc.gpsimd.tensor_reduce(m_loc[:, :], mp[:, :], axis=AX.C, op=OP.max)
            nm = wp.tile([1, 1], F32)
            nc.vector.tensor_scalar(out=nm[:, :], in0=m_loc[:, :], scalar1=-1.0, scalar2=None, op0=OP.mult)
            nmb_ps = ps.tile([128, 1], F32, tag="pss", bufs=3)
            nc.tensor.matmul(nmb_ps[:, :], ones128[:, :], nm[:, :], start=True, stop=True)
            nmb = wp.tile([128, 1], F32)
            nc.vector.tensor_copy(nmb[:, :], nmb_ps[:, :])
            e_sb = wp.tile([128, FP], F32)
            sp = wp.tile([128, 1], F32)
            nc.scalar.activation(e_sb[:, :], l_sb[:, :], AF.Exp, bias=nmb[:, :1], accum_out=sp[:, :1])
            s_loc = wp.tile([1, 1], F32)
            nc.gpsimd.tensor_reduce(s_loc[:, :], sp[:, :], axis=AX.C, op=OP.add)
            st = wp.tile([1, 2], F32)
            nc.vector.tensor_copy(st[:, 0:1], m_loc[:, :])
            nc.vector.tensor_copy(st[:, 1:2], s_loc[:, :])
            ags_in = dp.tile([2], F32)
            ags_out = dp.tile([2 * NCORES], F32)
            nc.sync.dma_start(row1(ags_in), st[:, :])
            nc.gpsimd.collective_compute(
                "AllGather", OP.bypass, replica_groups=rg,
                ins=[ags_in[:].opt()], outs=[ags_out[:].opt()],
            )
            mv = wp.tile([1, NCORES], F32)
            sv = wp.tile([1, NCORES], F32)
            ags_v = ags_out.rearrange("(c s) -> s c", s=2)
            nc.sync.dma_start(mv[:, :], ags_v[0:1, :])
            nc.sync.dma_start(sv[:, :], ags_v[1:2, :])
            mg = wp.tile([1, 1], F32)
            nc.vector.reduce_max(mg[:, :], mv[:, :], axis=AX.X)
            dm = wp.tile([1, NCORES], F32)
            nc.vector.tensor_scalar(out=dm[:, :], in0=mv[:, :], scalar1=mg[:, :1], scalar2=None, op0=OP.subtract)
            em = wp.tile([1, NCORES], F32)
            nc.scalar.activation(em[:, :], dm[:, :], AF.Exp)
            pr = wp.tile([1, NCORES], F32)
            nc.vector.tensor_mul(pr[:, :], em[:, :], sv[:, :])
            sg = wp.tile([1, 1], F32)
            nc.vector.reduce_sum(sg[:, :], pr[:, :], axis=AX.X)
            lng = wp.tile([1, 1], F32)
            nc.scalar.activation(lng[:, :], sg[:, :], AF.Ln)
            nlse = wp.tile([1, 1], F32)
            nc.vector.tensor_add(nlse[:, :], mg[:, :], lng[:, :])
            nc.vector.tensor_scalar(out=nlse[:, :], in0=nlse[:, :], scalar1=-1.0, scalar2=None, op0=OP.mult)
            nlb_ps = ps.tile([128, 1], F32, tag="pss", bufs=3)
            nc.tensor.matmul(nlb_ps[:, :], ones128[:, :], nlse[:, :], start=True, stop=True)
            nlb = wp.tile([128, 1], F32)
            nc.vector.tensor_copy(nlb[:, :], nlb_ps[:, :])
            logp_sb = wp.tile([128, FP], F32)
            nc.vector.tensor_scalar(out=logp_sb[:, :], in0=l_sb[:, :], scalar1=nlb[:, :1],
                                    scalar2=None, op0=OP.add)
            nc.sync.dma_start(logp_o[:, :], logp_sb[:, :])

    nc.compile()
    return nc


def _get_nc():
    global _CACHED_NC
    if _CACHED_NC is None:
        _CACHED_NC = _build()
    return _CACHED_NC


def kernel(input_tok, hidden, encoder_outputs, embedding_w,
           attn_combine_w, attn_combine_b, w_ih, w_hh, b_ih, b_hh,
           out_w, out_b):
    nc = _get_nc()

    tok = np.asarray(input_tok).astype(np.int32).reshape(1)
    h0 = np.asarray(hidden, dtype=np.float32).reshape(1, H)
    enc = np.ascontiguousarray(np.asarray(encoder_outputs, dtype=np.float32).reshape(L, H))
    encT = np.ascontiguousarray(enc.T)
    emb = np.asarray(embedding_w, dtype=np.float32)
    wc = np.asarray(attn_combine_w, dtype=np.float32)
    bc = np.asarray(attn_combine_b, dtype=np.float32)
    wih = np.asarray(w_ih, dtype=np.float32)
    whh = np.asarray(w_hh, dtype=np.float32)
    bi = np.asarray(b_ih, dtype=np.float32).reshape(3, HC, 128)
    bh = np.asarray(b_hh, dtype=np.float32).reshape(3, HC, 128)
    ow = np.asarray(out_w, dtype=np.float32)
    ob = np.asarray(out_b, dtype=np.float32)

    w_pad = np.zeros((NCORES * VP, H), dtype=np.float32)
    w_pad[:V] = ow
    b_pad = np.full(NCORES * VP, PAD_BIAS, dtype=np.float32)
    b_pad[:V] = ob
    h0cols = np.ascontiguousarray(h0.reshape(HC, 128).T)
    pack = lambda a, n: np.ascontiguousarray(
        a.reshape(n, 128, -1).transpose(1, 0, 2).reshape(128, -1))
    encp = pack(enc, 4)
    encTp = pack(encT, HC)
    # appl-half combine weight, block (k,j) = wcA.T[k*128:(k+1)*128, j*128:(j+1)*128]
    wcaT = np.ascontiguousarray(wc[:, H:].T)                       # [1024 appl-dims, 1024 out]
    wcap = np.ascontiguousarray(
        wcaT.reshape(HC, 128, HC, 128).transpose(1, 0, 2, 3).reshape(128, -1))

    in_maps = []
    for c in range(NCORES):
        sl = slice(c * 128, (c + 1) * 128)
        perm = [c ^ SIG[j] for j in range(HC)]
        wih_c = np.concatenate([wih[g * H + c * 128: g * H + (c + 1) * 128] for g in range(3)])
        whh_c = np.concatenate([whh[g * H + c * 128: g * H + (c + 1) * 128] for g in range(3)])
        wihp = pack(np.ascontiguousarray(wih_c.T), HC)
        whhp = pack(np.ascontiguousarray(whh_c.T), HC)
        wtT_c = np.ascontiguousarray(w_pad[c * VP:(c + 1) * VP].T).reshape(HC, 128, VP)
        wt_c = wtT_c[perm].reshape(H, VP).astype(ml_dtypes.bfloat16)
        vpack = np.zeros((128, VC_W), dtype=np.float32)
        vpack[:, VC_H0:VC_H0 + 8] = h0cols
        vpack[:, VC_BCC:VC_BCC + 8] = bc.reshape(HC, 128).T
        vpack[:, VC_OBT:VC_OBT + FP] = b_pad[c * VP:(c + 1) * VP].reshape(128, FP)
        rpack = np.zeros(RC_W, dtype=np.float32)
        rpack[RC_BRZ:RC_BRZ + 128] = bi[0, c] + bh[0, c]
        rpack[RC_BRZ + 128:RC_BRZ + 256] = bi[1, c] + bh[1, c]
        rpack[RC_BNIH:RC_BNIH + 128] = bi[2, c]
        rpack[RC_BNHH:RC_BNHH + 128] = bh[2, c]
        rpack[RC_H0S:RC_H0S + 128] = h0[0, sl]
        in_maps.append({
            "ipack": tok,
            "embs": np.ascontiguousarray(emb[:, sl]),
            "encp": encp,
            "encTp": encTp,
            "wcep": np.ascontiguousarray(wc[:, sl].T),             # [128 emb-dims, 1024 out]
            "wcap": wcap,
            "wihp": wihp,
            "whhp": whhp,
            "vpack": vpack,
            "rpack": rpack,
            "wt": wt_c,
        })

    global _last_in_maps
    _last_in_maps = in_maps
    res = run_bass_kernel_spmd(nc, in_maps, core_ids=list(range(NCORES)))

    logp = np.concatenate([res.results[c]["logp_o"].reshape(-1) for c in range(NCORES)])[:V]
    logp = logp.reshape(1, V).astype(np.float32)
    hid_raw = res.results[0]["hid_o"]                              # [128, 8], block j = chunk SIG[j]
    h_new = np.empty((HC, 128), dtype=np.float32)
    for j in range(HC):
        h_new[SIG[j]] = hid_raw[:, j]
    h_new = h_new.reshape(1, 1, H)
    attnw = res.results[0]["attnw_o"].reshape(1, L).astype(np.float32)
    return logp, h_new, attnw
